# revision 9
# baseline (speedup 1.0000x reference)
"""MultiHeadSeqAttention (adaptive-span sliding-window attention) Trainium2 kernel.

Problem (hardcoded shapes):
  B=8, M=512 (block), L=1024 (span limit), H=512, K=8 heads, D=64.
  query [8,512,512], key/value [8,1536,512], key_pe [1,64,1024],
  Wq/Wk/Wv/Wo [512,512], span_val [8,1,1].

Semantics (per batch b, head k):
  q = heads(query @ Wq.T), k/v likewise on key/value (length 1536 = M+L)
  attn[m, j] = softmax_j( (q[m].k[m+j] + q[m].pe[:, j]) * D**-0.5 ) * span_mask[j]
  out[m] = sum_j attn[m, j] * v[m+j],  j in [0, 1024)
  output = concat_heads(out) @ Wo.T

Sharding: data-parallel over batch; core b computes batch b entirely.

Device pipeline (per core), all matmuls bf16, fp32 PSUM:
  - Q^T = WqT-proj, K^T = WkT-proj (head-dim on partitions), V (key-pos on
    partitions, with a fused ones-column per head for softmax denominators).
  - Positional factor E_rel = exp(scale * q.pe) per head, written to a DRAM
    buffer with row stride 1153 (1024 data + 129 host-zeroed gap), then read
    back with row stride 1152 through the xbar transpose DMA: one DMA does
    unskew (relative->absolute coords) + transpose + exact band masking
    (out-of-band reads land in the zero gaps).
  - S^T[n, m] per 128-key chunk via PE; C = exp(scale*S) on ScalarE;
    P^T = C * E^T on VectorE; PV accumulates over 12 chunks with the ones
    column producing denominators; normalize; output projection in
    transposed layout (host transposes the [H, M] result back).
"""

import numpy as np
import ml_dtypes

B, M, L = 8, 512, 1024
MPL = M + L            # 1536
H, K, D = 512, 8, 64
SCALE = 1.0 / np.sqrt(D)
RAMP = 32.0
NCHUNK = MPL // 128    # 12 key chunks
NMT = M // 128         # 4 m-tiles
ES = L + 129           # 1153: skew storage row stride (elements)
EB = M * ES            # per-head skew buffer elements (590336)

BF16 = ml_dtypes.bfloat16

_cache = {}


def _mrange(w):
    """Query columns with any in-band key in chunk w (band: 0 <= n-m < 1024)."""
    return max(0, 128 * (w - 8)), min(M, 128 * (w + 1))


# staircase read rectangles (w0, w1, m0, m1) covering the band tightly;
# everything read outside the band lands in zeroed gap bytes.
_RECTS = [(0, 1, 0, 128), (1, 2, 0, 256), (2, 3, 0, 384), (3, 9, 0, 512),
          (9, 10, 128, 512), (10, 11, 256, 512), (11, 12, 384, 512)]


def _build(with_span_mask):
    import concourse.bass as bass
    import concourse.mybir as mybir
    import concourse.tile as tile
    from concourse import bacc
    from concourse.ap import AP

    fp32 = mybir.dt.float32
    bf16 = mybir.dt.bfloat16
    Exp = mybir.ActivationFunctionType.Exp
    Mult = mybir.AluOpType.mult

    nc = bacc.Bacc("TRN2", target_bir_lowering=False, debug=False, num_devices=8)

    xq = nc.dram_tensor("xq", [H, M], bf16, kind="ExternalInput").ap()      # query^T
    xk = nc.dram_tensor("xk", [H, MPL], bf16, kind="ExternalInput").ap()    # key^T
    xv = nc.dram_tensor("xv", [H, MPL], bf16, kind="ExternalInput").ap()    # value^T
    wq = nc.dram_tensor("wq", [H, H], bf16, kind="ExternalInput").ap()      # Wq^T
    wk = nc.dram_tensor("wk", [H, H], bf16, kind="ExternalInput").ap()
    wv = nc.dram_tensor("wv", [H, H], bf16, kind="ExternalInput").ap()
    wo = nc.dram_tensor("wo", [H, H], bf16, kind="ExternalInput").ap()
    pe2 = nc.dram_tensor("pe2", [128, L], bf16, kind="ExternalInput").ap()  # key_pe dup rows
    ez_t = nc.dram_tensor("ez", [K * EB], bf16, kind="ExternalInput")       # zeroed skew bufs
    if with_span_mask:
        smask = nc.dram_tensor("smask", [128, K, L], bf16, kind="ExternalInput").ap()
    out_t = nc.dram_tensor("out", [H, M], fp32, kind="ExternalOutput").ap()  # O^T

    with tile.TileContext(nc) as tc:
        with (
            tc.tile_pool(name="persist", bufs=1) as pp,
            tc.tile_pool(name="erel", bufs=4) as erel_pool,
            tc.tile_pool(name="eT", bufs=2) as eT_pool,
            tc.tile_pool(name="cp", bufs=4) as c_pool,
            tc.tile_pool(name="pp2", bufs=4) as p_pool,
            tc.tile_pool(name="oput", bufs=2) as o_pool,
            tc.tile_pool(name="ps_proj", bufs=2, space="PSUM") as ps_proj_pool,
            tc.tile_pool(name="ps_s", bufs=2, space="PSUM") as ps_s_pool,
            tc.tile_pool(name="ps_pos", bufs=2, space="PSUM") as ps_pos_pool,
            tc.tile_pool(name="ps_pv", bufs=2, space="PSUM") as ps_pv_pool,
        ):
            # ---- persistent SBUF tensors ----
            s_xq = pp.tile([128, 4, M], bf16, tag="s_xq")
            s_xk = pp.tile([128, 4, MPL], bf16, tag="s_xk")
            s_xv = pp.tile([128, 4, MPL], bf16, tag="s_xv")
            s_wq = pp.tile([128, 4, H], bf16, tag="s_wq")
            s_wk = pp.tile([128, 4, H], bf16, tag="s_wk")
            s_wv = pp.tile([128, 4, H], bf16, tag="s_wv")
            s_wo = pp.tile([128, 4, H], bf16, tag="s_wo")
            s_pe = pp.tile([128, L], bf16, tag="s_pe")
            s_q = pp.tile([128, 4, M], bf16, tag="s_q")      # Q^T
            s_k = pp.tile([128, 4, MPL], bf16, tag="s_k")    # K^T
            s_v = pp.tile([128, NCHUNK, K * 65], bf16, tag="s_v")  # V + ones cols
            s_ho = pp.tile([128, 4, M], bf16, tag="s_ho")    # HO^T
            if with_span_mask:
                s_sm = pp.tile([128, K, L], bf16, tag="s_sm")

            def load2d(sb, dram, rows, cols):
                # dram [rows, cols] -> sbuf [128, rows//128, cols]
                nt = rows // 128
                src = AP(dram.tensor, 0, [[cols, 128], [128 * cols, nt], [1, cols]])
                nc.sync.dma_start(sb[:, :, :], src)

            load2d(s_xq, xq, H, M)
            load2d(s_xk, xk, H, MPL)
            load2d(s_xv, xv, H, MPL)
            load2d(s_wq, wq, H, H)
            load2d(s_wk, wk, H, H)
            load2d(s_wv, wv, H, H)
            load2d(s_wo, wo, H, H)
            nc.sync.dma_start(s_pe[:, :], pe2)
            if with_span_mask:
                nc.sync.dma_start(s_sm[:, :, :], smask)

            # ones columns of s_v (col 65h+64 per head)
            sv3 = s_v[:, :, :].rearrange("p w (k c) -> p w k c", c=65)
            nc.gpsimd.memset(sv3[:, :, :, 64:65], 1.0)

            # ---- projections ----
            def proj(dst, w_s, x_s, ncols, dst_strided=False):
                # dst^T[h, n] = sum_e W^T[e, h] * x^T[e, n]
                for ht in range(4):
                    for nc_i in range(ncols // 512):
                        psm = ps_proj_pool.tile([128, 512], fp32, tag="proj")
                        for e in range(4):
                            nc.tensor.matmul(
                                psm[:, :],
                                w_s[:, e, 128 * ht:128 * (ht + 1)],
                                x_s[:, e, 512 * nc_i:512 * (nc_i + 1)],
                                start=(e == 0), stop=(e == 3),
                            )
                        nc.vector.tensor_copy(dst[:, ht, 512 * nc_i:512 * (nc_i + 1)], psm[:, :])

            proj(s_q, s_wq, s_xq, M)
            proj(s_k, s_wk, s_xk, MPL)

            # V projection: V[n, h2] = sum_e value^T[e, n] * Wv^T[e, h2]
            for nt in range(NCHUNK):
                psm = ps_proj_pool.tile([128, 512], fp32, tag="proj")
                for e in range(4):
                    nc.tensor.matmul(
                        psm[:, :],
                        s_xv[:, e, 128 * nt:128 * (nt + 1)],
                        s_wv[:, e, :],
                        start=(e == 0), stop=(e == 3),
                    )
                # scatter 64-col head groups into 65-col groups (ones col preserved)
                nc.vector.tensor_copy(
                    s_v[:, nt, :].rearrange("p (k c) -> p k c", c=65)[:, :, 0:64],
                    psm[:, :].rearrange("p (k c) -> p k c", c=64),
                )

            # ---- per-head attention ----
            for h in range(K):
                pb = (h % 2) * 64     # partition base for this head's q/k rows
                ht = h // 2
                qh = s_q[pb:pb + 64, ht, :]              # [64, 512]
                kh = s_k[pb:pb + 64, ht, :]              # [64, 1536]
                peh = s_pe[pb:pb + 64, :]                # [64, 1024]

                # POS + E_rel + skew write
                for mt in range(NMT):
                    er = erel_pool.tile([128, L], bf16, tag="erel")
                    for half in range(2):
                        pos_ps = ps_pos_pool.tile([128, 512], fp32, tag="pos")
                        nc.tensor.matmul(
                            pos_ps[:, :],
                            s_q[pb:pb + 64, ht, 128 * mt:128 * (mt + 1)],
                            peh[:, 512 * half:512 * (half + 1)],
                            start=True, stop=True,
                        )
                        nc.scalar.activation(
                            er[:, 512 * half:512 * (half + 1)], pos_ps[:, :],
                            Exp, scale=float(SCALE))
                    if with_span_mask:
                        nc.vector.tensor_tensor(
                            er[:, :], er[:, :], s_sm[:, h, :], op=Mult,
                        )
                    dst = AP(ez_t, h * EB + 128 * mt * ES, [[ES, 128], [1, L]])
                    nc.sync.dma_start(dst, er[:, :])

                # E^T staircase reads (skew + transpose + band-mask in one DMA)
                eT = eT_pool.tile([128, NCHUNK, M], bf16, tag="eT")
                for (w0, w1, m0, m1) in _RECTS:
                    src = AP(ez_t, h * EB + m0 * (ES - 1) + 128 * w0,
                             [[ES - 1, m1 - m0], [1, (w1 - w0) * 128]])
                    nc.sync.dma_start_transpose(eT[:, w0:w1, m0:m1], src)
                # zero the out-of-staircase corners
                for w in range(NCHUNK):
                    m0, m1 = _mrange(w)
                    if m0 > 0:
                        nc.gpsimd.memset(eT[:, w, 0:m0], 0.0)
                    if m1 < M:
                        nc.gpsimd.memset(eT[:, w, m1:M], 0.0)

                # S^T chunks + exp + P = C*E + PV accumulate
                pv_ps = ps_pv_pool.tile([65, 512], fp32, tag="pv")
                for w in range(NCHUNK):
                    s_ps = ps_s_pool.tile([128, 512], fp32, tag="sT")
                    nc.tensor.matmul(
                        s_ps[:, :],
                        s_k[pb:pb + 64, ht, 128 * w:128 * (w + 1)],
                        qh,
                        start=True, stop=True,
                    )
                    m0, m1 = _mrange(w)
                    ct = c_pool.tile([128, 512], bf16, tag="cT")
                    nc.scalar.activation(ct[:, m0:m1], s_ps[:, m0:m1], Exp, scale=float(SCALE))
                    pt = p_pool.tile([128, 512], bf16, tag="pT")
                    nc.vector.tensor_tensor(pt[:, m0:m1], ct[:, m0:m1], eT[:, w, m0:m1], op=Mult)
                    if m0 > 0:
                        nc.gpsimd.memset(pt[:, 0:m0], 0.0)
                    if m1 < M:
                        nc.gpsimd.memset(pt[:, m1:M], 0.0)
                    nc.tensor.matmul(
                        pv_ps[:, :],
                        s_v[:, w, 65 * h:65 * (h + 1)],
                        pt[:, :],
                        start=(w == 0), stop=(w == NCHUNK - 1),
                    )

                # normalize by denominator row (fused ones-column)
                den = o_pool.tile([1, 512], fp32, tag="den")
                nc.vector.reciprocal(den[:, :], pv_ps[64:65, :])
                denb = o_pool.tile([64, 512], fp32, tag="denb")
                nc.gpsimd.partition_broadcast(denb[:, :], den[:, :])
                nc.vector.tensor_tensor(
                    s_ho[pb:pb + 64, ht, :], pv_ps[0:64, :], denb[:, :], op=Mult,
                )

            # ---- output projection: O^T[h2, m] = sum_e Wo^T[e, h2] HO^T[e, m] ----
            for ht in range(4):
                psm = ps_proj_pool.tile([128, 512], fp32, tag="proj")
                for e in range(4):
                    nc.tensor.matmul(
                        psm[:, :],
                        s_wo[:, e, 128 * ht:128 * (ht + 1)],
                        s_ho[:, e, :],
                        start=(e == 0), stop=(e == 3),
                    )
                ot = o_pool.tile([128, 512], fp32, tag="ot")
                nc.vector.tensor_copy(ot[:, :], psm[:, :])
                nc.sync.dma_start(out_t[128 * ht:128 * (ht + 1), :], ot[:, :])

    nc.compile()
    return nc


def _prep_inputs(query, key, value, key_pe, Wq, Wk, Wv, Wo, span_val):
    """Host-side marshaling: transpose/cast/shard. Returns (in_maps, span_one)."""
    ez = np.zeros(K * EB, dtype=BF16)
    pe2 = np.concatenate([key_pe[0], key_pe[0]], axis=0).astype(BF16)  # [128, 1024]
    wqT = np.ascontiguousarray(Wq.T).astype(BF16)
    wkT = np.ascontiguousarray(Wk.T).astype(BF16)
    wvT = np.ascontiguousarray(Wv.T).astype(BF16)
    woT = np.ascontiguousarray(Wo.T).astype(BF16)

    template = np.linspace(1.0 - L, 0.0, L, dtype=np.float64)
    mask = np.clip((template[None, :] + span_val.reshape(K, 1).astype(np.float64) * L)
                   / RAMP + 1.0, 0.0, 1.0)
    span_one = bool(np.all(mask == 1.0))
    smask = None
    if not span_one:
        smask = np.broadcast_to(mask.astype(BF16)[None], (128, K, L)).copy()

    in_maps = []
    for b in range(B):
        m = {
            "xq": np.ascontiguousarray(query[b].T).astype(BF16),
            "xk": np.ascontiguousarray(key[b].T).astype(BF16),
            "xv": np.ascontiguousarray(value[b].T).astype(BF16),
            "wq": wqT, "wk": wkT, "wv": wvT, "wo": woT,
            "pe2": pe2, "ez": ez,
        }
        if smask is not None:
            m["smask"] = smask
        in_maps.append(m)
    return in_maps, span_one


def kernel(query, key, value, key_pe, Wq, Wk, Wv, Wo, span_val):
    from concourse.bass_utils import run_bass_kernel_spmd

    query = np.asarray(query, dtype=np.float32)
    key = np.asarray(key, dtype=np.float32)
    value = np.asarray(value, dtype=np.float32)
    key_pe = np.asarray(key_pe, dtype=np.float32)
    span_val = np.asarray(span_val, dtype=np.float32)

    in_maps, span_one = _prep_inputs(
        query, key, value, key_pe,
        np.asarray(Wq, np.float32), np.asarray(Wk, np.float32),
        np.asarray(Wv, np.float32), np.asarray(Wo, np.float32), span_val)

    variant = not span_one
    if variant not in _cache:
        _cache[variant] = _build(variant)
    nc = _cache[variant]

    res = run_bass_kernel_spmd(nc, in_maps, core_ids=list(range(8)))
    out = np.stack([np.ascontiguousarray(res.results[b]["out"].T) for b in range(B)])
    return out.astype(np.float32)


# revision 10
# speedup vs baseline: 1.0297x; 1.0297x over previous
"""MultiHeadSeqAttention (adaptive-span sliding-window attention) Trainium2 kernel.

Problem (hardcoded shapes):
  B=8, M=512 (block), L=1024 (span limit), H=512, K=8 heads, D=64.
  query [8,512,512], key/value [8,1536,512], key_pe [1,64,1024],
  Wq/Wk/Wv/Wo [512,512], span_val [8,1,1].

Semantics (per batch b, head k):
  q = heads(query @ Wq.T), k/v likewise on key/value (length 1536 = M+L)
  attn[m, j] = softmax_j( (q[m].k[m+j] + q[m].pe[:, j]) * D**-0.5 ) * span_mask[j]
  out[m] = sum_j attn[m, j] * v[m+j],  j in [0, 1024)
  output = concat_heads(out) @ Wo.T

Sharding: data-parallel over batch; core b computes batch b entirely.

Device pipeline (per core), all matmuls bf16, fp32 PSUM:
  - Q^T = WqT-proj, K^T = WkT-proj (head-dim on partitions), V (key-pos on
    partitions, with a fused ones-column per head for softmax denominators).
  - Positional factor E_rel = exp(scale * q.pe) per head, written to a DRAM
    buffer with row stride 1153 (1024 data + 129 host-zeroed gap), then read
    back with row stride 1152 through the xbar transpose DMA: one DMA does
    unskew (relative->absolute coords) + transpose + exact band masking
    (out-of-band reads land in the zero gaps).
  - S^T[n, m] per 128-key chunk via PE; C = exp(scale*S) on ScalarE;
    P^T = C * E^T on VectorE; PV accumulates over 12 chunks with the ones
    column producing denominators; normalize; output projection in
    transposed layout (host transposes the [H, M] result back).
"""

import numpy as np
import ml_dtypes

B, M, L = 8, 512, 1024
MPL = M + L            # 1536
H, K, D = 512, 8, 64
SCALE = 1.0 / np.sqrt(D)
RAMP = 32.0
NCHUNK = MPL // 128    # 12 key chunks
NMT = M // 128         # 4 m-tiles
ES = L + 129           # 1153: skew storage row stride (elements)
EB = M * ES            # per-head skew buffer elements (590336)

BF16 = ml_dtypes.bfloat16

_cache = {}


def _mrange(w):
    """Query columns with any in-band key in chunk w (band: 0 <= n-m < 1024)."""
    return max(0, 128 * (w - 8)), min(M, 128 * (w + 1))


# staircase read rectangles (w0, w1, m0, m1) covering the band tightly;
# everything read outside the band lands in zeroed gap bytes.
_RECTS = [(0, 1, 0, 128), (1, 2, 0, 256), (2, 3, 0, 384), (3, 9, 0, 512),
          (9, 10, 128, 512), (10, 11, 256, 512), (11, 12, 384, 512)]


def _build(with_span_mask):
    import concourse.bass as bass
    import concourse.mybir as mybir
    import concourse.tile as tile
    from concourse import bacc
    from concourse.ap import AP

    fp32 = mybir.dt.float32
    bf16 = mybir.dt.bfloat16
    Exp = mybir.ActivationFunctionType.Exp
    Mult = mybir.AluOpType.mult

    nc = bacc.Bacc("TRN2", target_bir_lowering=False, debug=False, num_devices=8)

    xq = nc.dram_tensor("xq", [H, M], bf16, kind="ExternalInput").ap()      # query^T
    xk = nc.dram_tensor("xk", [H, MPL], bf16, kind="ExternalInput").ap()    # key^T
    xv = nc.dram_tensor("xv", [H, MPL], bf16, kind="ExternalInput").ap()    # value^T
    wq = nc.dram_tensor("wq", [H, H], bf16, kind="ExternalInput").ap()      # Wq^T
    wk = nc.dram_tensor("wk", [H, H], bf16, kind="ExternalInput").ap()
    wv = nc.dram_tensor("wv", [H, H], bf16, kind="ExternalInput").ap()
    wo = nc.dram_tensor("wo", [H, H], bf16, kind="ExternalInput").ap()
    pe2 = nc.dram_tensor("pe2", [128, L], bf16, kind="ExternalInput").ap()  # key_pe dup rows
    ez_t = nc.dram_tensor("ez", [K * EB], bf16, kind="ExternalInput")       # zeroed skew bufs
    if with_span_mask:
        smask = nc.dram_tensor("smask", [128, K, L], bf16, kind="ExternalInput").ap()
    out_t = nc.dram_tensor("out", [H, M], fp32, kind="ExternalOutput").ap()  # O^T

    with tile.TileContext(nc) as tc:
        with (
            tc.tile_pool(name="persist", bufs=1) as pp,
            tc.tile_pool(name="erel", bufs=4) as erel_pool,
            tc.tile_pool(name="eT", bufs=8) as eT_pool,
            tc.tile_pool(name="cp", bufs=4) as c_pool,
            tc.tile_pool(name="pp2", bufs=4) as p_pool,
            tc.tile_pool(name="oput", bufs=2) as o_pool,
            tc.tile_pool(name="ps_proj", bufs=2, space="PSUM") as ps_proj_pool,
            tc.tile_pool(name="ps_s", bufs=2, space="PSUM") as ps_s_pool,
            tc.tile_pool(name="ps_pos", bufs=2, space="PSUM") as ps_pos_pool,
            tc.tile_pool(name="ps_pv", bufs=2, space="PSUM") as ps_pv_pool,
        ):
            # ---- persistent SBUF tensors ----
            s_xq = pp.tile([128, 4, M], bf16, tag="s_xq")
            s_xk = pp.tile([128, 4, MPL], bf16, tag="s_xk")
            s_xv = pp.tile([128, 4, MPL], bf16, tag="s_xv")
            s_wq = pp.tile([128, 4, H], bf16, tag="s_wq")
            s_wk = pp.tile([128, 4, H], bf16, tag="s_wk")
            s_wv = pp.tile([128, 4, H], bf16, tag="s_wv")
            s_wo = pp.tile([128, 4, H], bf16, tag="s_wo")
            s_pe = pp.tile([128, L], bf16, tag="s_pe")
            s_q = pp.tile([128, 4, M], bf16, tag="s_q")      # Q^T
            s_k = pp.tile([128, 4, MPL], bf16, tag="s_k")    # K^T
            s_v = pp.tile([128, NCHUNK, K * 65], bf16, tag="s_v")  # V + ones cols
            s_ho = pp.tile([128, 4, M], bf16, tag="s_ho")    # HO^T
            if with_span_mask:
                s_sm = pp.tile([128, K, L], bf16, tag="s_sm")

            def load2d(sb, dram, rows, cols):
                # dram [rows, cols] -> sbuf [128, rows//128, cols]
                nt = rows // 128
                src = AP(dram.tensor, 0, [[cols, 128], [128 * cols, nt], [1, cols]])
                nc.sync.dma_start(sb[:, :, :], src)

            load2d(s_xq, xq, H, M)
            load2d(s_xk, xk, H, MPL)
            load2d(s_xv, xv, H, MPL)
            load2d(s_wq, wq, H, H)
            load2d(s_wk, wk, H, H)
            load2d(s_wv, wv, H, H)
            load2d(s_wo, wo, H, H)
            nc.sync.dma_start(s_pe[:, :], pe2)
            if with_span_mask:
                nc.sync.dma_start(s_sm[:, :, :], smask)

            # ones columns of s_v (col 65h+64 per head)
            sv3 = s_v[:, :, :].rearrange("p w (k c) -> p w k c", c=65)
            nc.gpsimd.memset(sv3[:, :, :, 64:65], 1.0)

            # ---- projections ----
            def proj(dst, w_s, x_s, ncols, dst_strided=False):
                # dst^T[h, n] = sum_e W^T[e, h] * x^T[e, n]
                for ht in range(4):
                    for nc_i in range(ncols // 512):
                        psm = ps_proj_pool.tile([128, 512], fp32, tag="proj")
                        for e in range(4):
                            nc.tensor.matmul(
                                psm[:, :],
                                w_s[:, e, 128 * ht:128 * (ht + 1)],
                                x_s[:, e, 512 * nc_i:512 * (nc_i + 1)],
                                start=(e == 0), stop=(e == 3),
                            )
                        nc.vector.tensor_copy(dst[:, ht, 512 * nc_i:512 * (nc_i + 1)], psm[:, :])

            proj(s_q, s_wq, s_xq, M)
            proj(s_k, s_wk, s_xk, MPL)

            # V projection: V[n, h2] = sum_e value^T[e, n] * Wv^T[e, h2]
            for nt in range(NCHUNK):
                psm = ps_proj_pool.tile([128, 512], fp32, tag="proj")
                for e in range(4):
                    nc.tensor.matmul(
                        psm[:, :],
                        s_xv[:, e, 128 * nt:128 * (nt + 1)],
                        s_wv[:, e, :],
                        start=(e == 0), stop=(e == 3),
                    )
                # scatter 64-col head groups into 65-col groups (ones col preserved)
                nc.vector.tensor_copy(
                    s_v[:, nt, :].rearrange("p (k c) -> p k c", c=65)[:, :, 0:64],
                    psm[:, :].rearrange("p (k c) -> p k c", c=64),
                )

            # ---- POS + E round trip for ALL heads first (deep pipeline) ----
            eTs = {}
            for h in range(K):
                pb = (h % 2) * 64
                ht = h // 2
                peh = s_pe[pb:pb + 64, :]

                for mt in range(NMT):
                    er = erel_pool.tile([128, L], bf16, tag="erel")
                    for half in range(2):
                        pos_ps = ps_pos_pool.tile([128, 512], fp32, tag="pos")
                        nc.tensor.matmul(
                            pos_ps[:, :],
                            s_q[pb:pb + 64, ht, 128 * mt:128 * (mt + 1)],
                            peh[:, 512 * half:512 * (half + 1)],
                            start=True, stop=True,
                        )
                        nc.scalar.activation(
                            er[:, 512 * half:512 * (half + 1)], pos_ps[:, :],
                            Exp, scale=float(SCALE))
                    if with_span_mask:
                        nc.vector.tensor_tensor(
                            er[:, :], er[:, :], s_sm[:, h, :], op=Mult,
                        )
                    dst = AP(ez_t, h * EB + 128 * mt * ES, [[ES, 128], [1, L]])
                    nc.sync.dma_start(dst, er[:, :])

                # E^T: one skew+transpose+band-mask DMA for the whole head
                eT = eT_pool.tile([128, NCHUNK, M], bf16, tag="eT")
                src_ap = AP(ez_t, h * EB, [[ES - 1, M], [1, MPL]])
                nc.sync.dma_start_transpose(eT[:, :, :], src_ap)
                # zero the out-of-band corners (overwrites garbage from the wide read)
                for w in range(NCHUNK):
                    m0, m1 = _mrange(w)
                    if m0 > 0:
                        nc.gpsimd.memset(eT[:, w, 0:m0], 0.0)
                    if m1 < M:
                        nc.gpsimd.memset(eT[:, w, m1:M], 0.0)
                eTs[h] = eT

            # ---- per-head attention ----
            for h in range(K):
                pb = (h % 2) * 64
                ht = h // 2
                qh = s_q[pb:pb + 64, ht, :]
                eT = eTs[h]

                pv_ps = ps_pv_pool.tile([65, 512], fp32, tag="pv")
                for w in range(NCHUNK):
                    s_ps = ps_s_pool.tile([128, 512], fp32, tag="sT")
                    nc.tensor.matmul(
                        s_ps[:, :],
                        s_k[pb:pb + 64, ht, 128 * w:128 * (w + 1)],
                        qh,
                        start=True, stop=True,
                    )
                    m0, m1 = _mrange(w)
                    ct = c_pool.tile([128, 512], bf16, tag="cT")
                    nc.scalar.activation(ct[:, m0:m1], s_ps[:, m0:m1], Exp, scale=float(SCALE))
                    pt = p_pool.tile([128, 512], bf16, tag="pT")
                    nc.vector.tensor_tensor(pt[:, m0:m1], ct[:, m0:m1], eT[:, w, m0:m1], op=Mult)
                    if m0 > 0:
                        nc.gpsimd.memset(pt[:, 0:m0], 0.0)
                    if m1 < M:
                        nc.gpsimd.memset(pt[:, m1:M], 0.0)
                    nc.tensor.matmul(
                        pv_ps[:, :],
                        s_v[:, w, 65 * h:65 * (h + 1)],
                        pt[:, :],
                        start=(w == 0), stop=(w == NCHUNK - 1),
                    )

                # normalize by denominator row (fused ones-column)
                den = o_pool.tile([1, 512], fp32, tag="den")
                nc.vector.tensor_copy(den[:, :], pv_ps[64:65, :])
                denb = o_pool.tile([64, 512], fp32, tag="denb")
                nc.gpsimd.partition_broadcast(denb[:, :], den[:, :])
                nc.vector.reciprocal(denb[:, :], denb[:, :])
                nc.vector.tensor_tensor(
                    s_ho[pb:pb + 64, ht, :], pv_ps[0:64, :], denb[:, :], op=Mult,
                )

            # ---- output projection: O^T[h2, m] = sum_e Wo^T[e, h2] HO^T[e, m] ----
            for ht in range(4):
                psm = ps_proj_pool.tile([128, 512], fp32, tag="proj")
                for e in range(4):
                    nc.tensor.matmul(
                        psm[:, :],
                        s_wo[:, e, 128 * ht:128 * (ht + 1)],
                        s_ho[:, e, :],
                        start=(e == 0), stop=(e == 3),
                    )
                ot = o_pool.tile([128, 512], fp32, tag="ot")
                nc.vector.tensor_copy(ot[:, :], psm[:, :])
                nc.sync.dma_start(out_t[128 * ht:128 * (ht + 1), :], ot[:, :])

    nc.compile()
    return nc


def _prep_inputs(query, key, value, key_pe, Wq, Wk, Wv, Wo, span_val):
    """Host-side marshaling: transpose/cast/shard. Returns (in_maps, span_one)."""
    ez = np.zeros(K * EB, dtype=BF16)
    pe2 = np.concatenate([key_pe[0], key_pe[0]], axis=0).astype(BF16)  # [128, 1024]
    wqT = np.ascontiguousarray(Wq.T).astype(BF16)
    wkT = np.ascontiguousarray(Wk.T).astype(BF16)
    wvT = np.ascontiguousarray(Wv.T).astype(BF16)
    woT = np.ascontiguousarray(Wo.T).astype(BF16)

    template = np.linspace(1.0 - L, 0.0, L, dtype=np.float64)
    mask = np.clip((template[None, :] + span_val.reshape(K, 1).astype(np.float64) * L)
                   / RAMP + 1.0, 0.0, 1.0)
    span_one = bool(np.all(mask == 1.0))
    smask = None
    if not span_one:
        smask = np.broadcast_to(mask.astype(BF16)[None], (128, K, L)).copy()

    in_maps = []
    for b in range(B):
        m = {
            "xq": np.ascontiguousarray(query[b].T).astype(BF16),
            "xk": np.ascontiguousarray(key[b].T).astype(BF16),
            "xv": np.ascontiguousarray(value[b].T).astype(BF16),
            "wq": wqT, "wk": wkT, "wv": wvT, "wo": woT,
            "pe2": pe2, "ez": ez,
        }
        if smask is not None:
            m["smask"] = smask
        in_maps.append(m)
    return in_maps, span_one


def kernel(query, key, value, key_pe, Wq, Wk, Wv, Wo, span_val):
    from concourse.bass_utils import run_bass_kernel_spmd

    query = np.asarray(query, dtype=np.float32)
    key = np.asarray(key, dtype=np.float32)
    value = np.asarray(value, dtype=np.float32)
    key_pe = np.asarray(key_pe, dtype=np.float32)
    span_val = np.asarray(span_val, dtype=np.float32)

    in_maps, span_one = _prep_inputs(
        query, key, value, key_pe,
        np.asarray(Wq, np.float32), np.asarray(Wk, np.float32),
        np.asarray(Wv, np.float32), np.asarray(Wo, np.float32), span_val)

    variant = not span_one
    if variant not in _cache:
        _cache[variant] = _build(variant)
    nc = _cache[variant]

    res = run_bass_kernel_spmd(nc, in_maps, core_ids=list(range(8)))
    out = np.stack([np.ascontiguousarray(res.results[b]["out"].T) for b in range(B)])
    return out.astype(np.float32)


# revision 16
# speedup vs baseline: 1.1540x; 1.1207x over previous
"""MultiHeadSeqAttention (adaptive-span sliding-window attention) Trainium2 kernel.

Problem (hardcoded shapes):
  B=8, M=512 (block), L=1024 (span limit), H=512, K=8 heads, D=64.
  query [8,512,512], key/value [8,1536,512], key_pe [1,64,1024],
  Wq/Wk/Wv/Wo [512,512], span_val [8,1,1].

Semantics (per batch b, head k):
  q = heads(query @ Wq.T), k/v likewise on key/value (length 1536 = M+L)
  attn[m, j] = softmax_j( (q[m].k[m+j] + q[m].pe[:, j]) * D**-0.5 ) * span_mask[j]
  out[m] = sum_j attn[m, j] * v[m+j],  j in [0, 1024)
  output = concat_heads(out) @ Wo.T

Sharding: data-parallel over batch; core b computes batch b entirely.

Device pipeline (per core), all matmuls bf16, fp32 PSUM:
  - Q^T = WqT-proj, K^T = WkT-proj (head-dim on partitions), V (key-pos on
    partitions, with a fused ones-column per head for softmax denominators).
  - Positional factor E_rel = exp(scale * q.pe) per head, written to a DRAM
    buffer with row stride 1153 (1024 data + 129 host-zeroed gap), then read
    back with row stride 1152 through the xbar transpose DMA: one DMA does
    unskew (relative->absolute coords) + transpose + exact band masking
    (out-of-band reads land in the zero gaps).
  - S^T[n, m] per 128-key chunk via PE; C = exp(scale*S) on ScalarE;
    P^T = C * E^T on VectorE; PV accumulates over 12 chunks with the ones
    column producing denominators; normalize; output projection in
    transposed layout (host transposes the [H, M] result back).
"""

import numpy as np
import ml_dtypes

B, M, L = 8, 512, 1024
MPL = M + L            # 1536
H, K, D = 512, 8, 64
SCALE = 1.0 / np.sqrt(D)
RAMP = 32.0
NCHUNK = MPL // 128    # 12 key chunks
NMT = M // 128         # 4 m-tiles
ES = L + 129           # 1153: skew storage row stride (elements)
EB = M * ES            # per-head skew buffer elements (590336)

BF16 = ml_dtypes.bfloat16

_cache = {}


def _mrange(w):
    """Query columns with any in-band key in chunk w (band: 0 <= n-m < 1024)."""
    return max(0, 128 * (w - 8)), min(M, 128 * (w + 1))


# staircase read rectangles (w0, w1, m0, m1) covering the band tightly;
# everything read outside the band lands in zeroed gap bytes.
_RECTS = [(0, 1, 0, 128), (1, 2, 0, 256), (2, 3, 0, 384), (3, 9, 0, 512),
          (9, 10, 128, 512), (10, 11, 256, 512), (11, 12, 384, 512)]


def _build(with_span_mask):
    import concourse.bass as bass
    import concourse.mybir as mybir
    import concourse.tile as tile
    from concourse import bacc
    from concourse.ap import AP

    fp32 = mybir.dt.float32
    bf16 = mybir.dt.bfloat16
    Exp = mybir.ActivationFunctionType.Exp
    Mult = mybir.AluOpType.mult

    nc = bacc.Bacc("TRN2", target_bir_lowering=False, debug=False, num_devices=8)

    xq = nc.dram_tensor("xq", [H, M], bf16, kind="ExternalInput").ap()      # query^T
    xk = nc.dram_tensor("xk", [H, MPL], bf16, kind="ExternalInput").ap()    # key^T
    xv = nc.dram_tensor("xv", [H, MPL], bf16, kind="ExternalInput").ap()    # value^T
    wq = nc.dram_tensor("wq", [H, H], bf16, kind="ExternalInput").ap()      # Wq^T
    wk = nc.dram_tensor("wk", [H, H], bf16, kind="ExternalInput").ap()
    wv = nc.dram_tensor("wv", [H, H], bf16, kind="ExternalInput").ap()
    wo = nc.dram_tensor("wo", [H, H], bf16, kind="ExternalInput").ap()
    pe2 = nc.dram_tensor("pe2", [128, L], bf16, kind="ExternalInput").ap()  # key_pe dup rows
    ez_t = nc.dram_tensor("ez", [K * EB], bf16, kind="ExternalInput")       # zeroed skew bufs
    if with_span_mask:
        smask = nc.dram_tensor("smask", [128, K, L], bf16, kind="ExternalInput").ap()
    out_t = nc.dram_tensor("out", [H, M], fp32, kind="ExternalOutput").ap()  # O^T

    with tile.TileContext(nc) as tc:
        with (
            tc.tile_pool(name="persist", bufs=1) as pp,
            tc.tile_pool(name="erel", bufs=4) as erel_pool,
            tc.tile_pool(name="eT", bufs=6) as eT_pool,
            tc.tile_pool(name="cp", bufs=6) as c_pool,
            tc.tile_pool(name="pp2", bufs=13) as p_pool,
            tc.tile_pool(name="oput", bufs=2) as o_pool,
            tc.tile_pool(name="ps_a", bufs=2, space="PSUM") as ps_a_pool,
            tc.tile_pool(name="ps_s", bufs=4, space="PSUM") as ps_s_pool,
            tc.tile_pool(name="ps_pv", bufs=2, space="PSUM") as ps_pv_pool,
        ):
            # ---- persistent SBUF tensors ----
            s_xq = pp.tile([128, 4, M], bf16, tag="s_xq")
            s_xk = pp.tile([128, 4, MPL], bf16, tag="s_xk")
            s_xv = pp.tile([128, 4, MPL], bf16, tag="s_xv")
            s_wq = pp.tile([128, 4, H], bf16, tag="s_wq")
            s_wk = pp.tile([128, 4, H], bf16, tag="s_wk")
            s_wv = pp.tile([128, 4, H], bf16, tag="s_wv")
            s_wo = pp.tile([128, 4, H], bf16, tag="s_wo")
            s_pe = pp.tile([128, L], bf16, tag="s_pe")
            s_q = pp.tile([128, 4, M], bf16, tag="s_q")      # Q^T
            s_k = pp.tile([128, 4, MPL], bf16, tag="s_k")    # K^T
            s_v = pp.tile([128, NCHUNK, K * 65], bf16, tag="s_v")  # V + ones cols
            s_ho = pp.tile([128, 4, M], bf16, tag="s_ho")    # HO^T
            if with_span_mask:
                s_sm = pp.tile([128, K, L], bf16, tag="s_sm")

            def load2d(sb, dram, rows, cols):
                # dram [rows, cols] -> sbuf [128, rows//128, cols]
                nt = rows // 128
                src = AP(dram.tensor, 0, [[cols, 128], [128 * cols, nt], [1, cols]])
                nc.sync.dma_start(sb[:, :, :], src)

            load2d(s_xq, xq, H, M)
            load2d(s_xk, xk, H, MPL)
            load2d(s_xv, xv, H, MPL)
            load2d(s_wq, wq, H, H)
            load2d(s_wk, wk, H, H)
            load2d(s_wv, wv, H, H)
            load2d(s_wo, wo, H, H)
            nc.sync.dma_start(s_pe[:, :], pe2)
            if with_span_mask:
                nc.sync.dma_start(s_sm[:, :, :], smask)

            # ones columns of s_v (col 65h+64 per head)
            sv3 = s_v[:, :, :].rearrange("p w (k c) -> p w k c", c=65)
            nc.gpsimd.memset(sv3[:, :, :, 64:65], 1.0)

            # ---- projections ----
            def proj(dst, w_s, x_s, ncols, dst_strided=False):
                # dst^T[h, n] = sum_e W^T[e, h] * x^T[e, n]
                for ht in range(4):
                    for nc_i in range(ncols // 512):
                        psm = ps_a_pool.tile([128, 512], fp32, tag="work")
                        for e in range(4):
                            nc.tensor.matmul(
                                psm[:, :],
                                w_s[:, e, 128 * ht:128 * (ht + 1)],
                                x_s[:, e, 512 * nc_i:512 * (nc_i + 1)],
                                start=(e == 0), stop=(e == 3),
                            )
                        nc.vector.tensor_copy(dst[:, ht, 512 * nc_i:512 * (nc_i + 1)], psm[:, :])

            proj(s_q, s_wq, s_xq, M)
            proj(s_k, s_wk, s_xk, MPL)

            # V projection: V[n, h2] = sum_e value^T[e, n] * Wv^T[e, h2]
            for nt in range(NCHUNK):
                psm = ps_a_pool.tile([128, 512], fp32, tag="work")
                for e in range(4):
                    nc.tensor.matmul(
                        psm[:, :],
                        s_xv[:, e, 128 * nt:128 * (nt + 1)],
                        s_wv[:, e, :],
                        start=(e == 0), stop=(e == 3),
                    )
                # scatter 64-col head groups into 65-col groups (ones col preserved)
                nc.vector.tensor_copy(
                    s_v[:, nt, :].rearrange("p (k c) -> p k c", c=65)[:, :, 0:64],
                    psm[:, :].rearrange("p (k c) -> p k c", c=64),
                )

            # ---- POS + E round trip for ALL heads first (deep pipeline) ----
            eTs = {}
            for hp in range(K // 2):
                for mt in range(NMT):
                    ers = {}
                    for sub in range(2):           # even/odd head of the pair
                        h = 2 * hp + sub
                        pb = sub * 64
                        ers[h] = erel_pool.tile([128, L], bf16, tag="erel", name=f"er_{h}_{mt}")
                    for half in range(2):
                        pps = {}
                        for sub in range(2):       # adjacent issue -> concurrent rows
                            h = 2 * hp + sub
                            pb = sub * 64
                            pos_ps = ps_a_pool.tile([128, 512], fp32, tag="work")
                            nc.tensor.matmul(
                                pos_ps[:, :],
                                s_q[pb:pb + 64, hp, 128 * mt:128 * (mt + 1)],
                                s_pe[pb:pb + 64, 512 * half:512 * (half + 1)],
                                start=True, stop=True,
                            )
                            pps[h] = pos_ps
                        for sub in range(2):
                            h = 2 * hp + sub
                            nc.scalar.activation(
                                ers[h][:, 512 * half:512 * (half + 1)], pps[h][:, :],
                                Exp, scale=float(SCALE))
                    for sub in range(2):
                        h = 2 * hp + sub
                        if with_span_mask:
                            nc.vector.tensor_tensor(
                                ers[h][:, :], ers[h][:, :], s_sm[:, h, :], op=Mult,
                            )
                        dst = AP(ez_t, h * EB + 128 * mt * ES, [[ES, 128], [1, L]])
                        nc.sync.dma_start(dst, ers[h][:, :])

                # E^T: one skew+transpose+band-mask DMA per head of the pair
                for sub in range(2):
                    h = 2 * hp + sub
                    eT = eT_pool.tile([128, NCHUNK, M], bf16, tag="eT",
                                      name=f"eT_{h}")
                    src_ap = AP(ez_t, h * EB, [[ES - 1, M], [1, MPL]])
                    nc.sync.dma_start_transpose(eT[:, :, :], src_ap)
                    # zero out-of-band corners (overwrites garbage from wide read)
                    for w in range(NCHUNK):
                        m0, m1 = _mrange(w)
                        if m0 > 0:
                            nc.gpsimd.memset(eT[:, w, 0:m0], 0.0)
                        if m1 < M:
                            nc.gpsimd.memset(eT[:, w, m1:M], 0.0)
                    eTs[h] = eT

            # ---- per-head attention ----
            for h in range(K):
                pb = (h % 2) * 64
                ht = h // 2
                qh = s_q[pb:pb + 64, ht, :]
                eT = eTs[h]

                pv_ps = ps_pv_pool.tile([65, 512], fp32, tag="pv")
                pts = []
                for w in range(NCHUNK):
                    m0, m1 = _mrange(w)
                    s_ps = ps_s_pool.tile([128, 512], fp32, tag="sT")
                    nc.tensor.matmul(
                        s_ps[:, m0:m1],
                        s_k[pb:pb + 64, ht, 128 * w:128 * (w + 1)],
                        qh[:, m0:m1],
                        start=True, stop=True,
                    )
                    ct = c_pool.tile([128, 512], bf16, tag="cT")
                    nc.scalar.activation(ct[:, m0:m1], s_ps[:, m0:m1], Exp, scale=float(SCALE))
                    pt = p_pool.tile([128, 512], bf16, tag="pT")
                    nc.vector.tensor_tensor(pt[:, m0:m1], ct[:, m0:m1], eT[:, w, m0:m1], op=Mult)
                    if m0 > 0:
                        nc.gpsimd.memset(pt[:, 0:m0], 0.0)
                    if m1 < M:
                        nc.gpsimd.memset(pt[:, m1:M], 0.0)
                    pts.append(pt)
                for w in range(NCHUNK):
                    nc.tensor.matmul(
                        pv_ps[:, :],
                        s_v[:, w, 65 * h:65 * (h + 1)],
                        pts[w][:, :],
                        start=(w == 0), stop=(w == NCHUNK - 1),
                    )

                # normalize via fast approx reciprocal on 64 lanes
                den = o_pool.tile([1, 512], fp32, tag="den")
                nc.vector.tensor_copy(den[:, :], pv_ps[64:65, :])
                denb = o_pool.tile([64, 512], fp32, tag="denb")
                nc.gpsimd.partition_broadcast(denb[:, :], den[:, :])
                rec = o_pool.tile([64, 512], fp32, tag="rec")
                nc.vector.reciprocal_approx_fast(rec[:, :], denb[:, :])
                nc.vector.tensor_tensor(
                    s_ho[pb:pb + 64, ht, :], pv_ps[0:64, :], rec[:, :], op=Mult,
                )

            # ---- output projection: O^T[h2, m] = sum_e Wo^T[e, h2] HO^T[e, m] ----
            for ht in range(4):
                psm = ps_a_pool.tile([128, 512], fp32, tag="work")
                for e in range(4):
                    nc.tensor.matmul(
                        psm[:, :],
                        s_wo[:, e, 128 * ht:128 * (ht + 1)],
                        s_ho[:, e, :],
                        start=(e == 0), stop=(e == 3),
                    )
                ot = o_pool.tile([128, 512], fp32, tag="ot")
                nc.vector.tensor_copy(ot[:, :], psm[:, :])
                nc.sync.dma_start(out_t[128 * ht:128 * (ht + 1), :], ot[:, :])

    nc.compile()
    return nc


def _prep_inputs(query, key, value, key_pe, Wq, Wk, Wv, Wo, span_val):
    """Host-side marshaling: transpose/cast/shard. Returns (in_maps, span_one)."""
    ez = np.zeros(K * EB, dtype=BF16)
    pe2 = np.concatenate([key_pe[0], key_pe[0]], axis=0).astype(BF16)  # [128, 1024]
    wqT = np.ascontiguousarray(Wq.T).astype(BF16)
    wkT = np.ascontiguousarray(Wk.T).astype(BF16)
    wvT = np.ascontiguousarray(Wv.T).astype(BF16)
    woT = np.ascontiguousarray(Wo.T).astype(BF16)

    template = np.linspace(1.0 - L, 0.0, L, dtype=np.float64)
    mask = np.clip((template[None, :] + span_val.reshape(K, 1).astype(np.float64) * L)
                   / RAMP + 1.0, 0.0, 1.0)
    span_one = bool(np.all(mask == 1.0))
    smask = None
    if not span_one:
        smask = np.broadcast_to(mask.astype(BF16)[None], (128, K, L)).copy()

    in_maps = []
    for b in range(B):
        m = {
            "xq": np.ascontiguousarray(query[b].T).astype(BF16),
            "xk": np.ascontiguousarray(key[b].T).astype(BF16),
            "xv": np.ascontiguousarray(value[b].T).astype(BF16),
            "wq": wqT, "wk": wkT, "wv": wvT, "wo": woT,
            "pe2": pe2, "ez": ez,
        }
        if smask is not None:
            m["smask"] = smask
        in_maps.append(m)
    return in_maps, span_one


def kernel(query, key, value, key_pe, Wq, Wk, Wv, Wo, span_val):
    from concourse.bass_utils import run_bass_kernel_spmd

    query = np.asarray(query, dtype=np.float32)
    key = np.asarray(key, dtype=np.float32)
    value = np.asarray(value, dtype=np.float32)
    key_pe = np.asarray(key_pe, dtype=np.float32)
    span_val = np.asarray(span_val, dtype=np.float32)

    in_maps, span_one = _prep_inputs(
        query, key, value, key_pe,
        np.asarray(Wq, np.float32), np.asarray(Wk, np.float32),
        np.asarray(Wv, np.float32), np.asarray(Wo, np.float32), span_val)

    variant = not span_one
    if variant not in _cache:
        _cache[variant] = _build(variant)
    nc = _cache[variant]

    res = run_bass_kernel_spmd(nc, in_maps, core_ids=list(range(8)))
    out = np.stack([np.ascontiguousarray(res.results[b]["out"].T) for b in range(B)])
    return out.astype(np.float32)
